# revision 19
# baseline (speedup 1.0000x reference)
"""Trainium2 Bass kernel for nn_CalibrationNetwork (MoE-routed 3-layer MLP + softmax).

Strategy
--------
Host side (numpy): sort samples by judge id, pad each judge group to a
multiple of 256 samples ("supertile"), distribute supertiles round-robin
over 8 cores (20 slots/core covers the worst case sum_j ceil(c_j/256) <= 160).
All judge-specific tables are combined with the shared ones on the host
(W1+W1_a[j] etc.), so the device never gathers. Questions are processed as
block-diagonal pairs (two 64-wide blocks fill the 128 partitions/PE columns);
the odd 7th question (q6) of the two supertiles in a macro-iteration shares
one block (s=0 half = first tile's q6, s=1 half = second tile's q6, with the
two tiles' experts mixed in the block-diagonal weights).

Per supertile the host emits, merged into two HBM streams:
  - xw [12, XW] bf16: cols 0..1024 = transposed x with bias-ones rows (rows
    s*6+d; d=0 is ones), pair p in cols p*256; cols 1024..1616 = L1
    stationary blocks (rows s*6+d, cols p*128+s*64+h) with partition-0 cols
    1536..1696 holding both tiles' L3 bias rows b3 (o-major, (blk,p,s,o)).
  - wb [128, 300] bf16: cols 0..128 L2 block-diag stationary [[W2,0],[0,W2]];
    cols 128..168 L3 moving operand (V block per pair, cols p*10+s*5+o);
    cols 168..172 the L2 bias columns as raw f32 bits (bitcast on device,
    own bias then q6-mixed bias); cols 172..300 the q6-mixed L2 stationary
    [[W2(j0),0],[0,W2(j1)]].

Device (per macro = two supertiles; all engines pipelined by Tile):
  bias: 1 matmul ones_col^T @ b3rows -> psum3 [128, 160]        (K=1, N=160)
  L1: 7 matmuls  psum1[(s,h1), (p,b)] += wa_p^T @ xt_p          (K=12, M=128, N=256)
  relu1 (DVE):   z1 = max(psum1, 0) -> bf16                     (bias via ones row)
  L2: 7 matmuls  psum2[(s,h2), (p,b)] += w2blk^T @ z1_p         (K=128, M=128, N=256)
  relu2 (ACT):   z2 = relu(psum2 + b2)  -> bf16                 (per-partition bias)
  L3: 16 matmuls psum3[b, cols] += z2_slice^T @ vb_p            (K=128, M=128, N=5..10)
  softmax: exp (ACT), grouped reduce_sum + reciprocal (DVE), multiply (GpSimd)
  DMA: batched per macro - xw/wb loads on the SP HWDGE ring, the [128, 160]
  f32 output store on the GpSimd SWDGE queue; host strips padding + unsorts.

The program is compiled per call for the actual supertile count
(rounded to whole macros) rather than the worst-case T=20; with the
reference inputs that is 18 slots/core (9 macros).
Measured on 8 axon NeuronCores: ~60.5 us HW exec, absmax err ~1.1e-3.
"""

import numpy as np
import ml_dtypes

B, J, Q, O = 32768, 32, 7, 5
H = 64            # H1 == H2
ST = 256          # samples per supertile
T = 20            # supertiles per core (worst case 32768/256 + 32 = 160 = 8*20)
N_CORES = 8
QP = 4            # question pairs (Q=7 padded to 8)
XSW = 928           # xs row width: 2*256 xt | 2*128 wa | 160 b3 (row 0)
WBW = 300           # per-tile wb width (172 + mixed-W2 block 128)

_bf16 = ml_dtypes.bfloat16
_cache = {}


# ----------------------------------------------------------------------------
# device program
# ----------------------------------------------------------------------------

def _build_program(teff):
    import concourse.bacc as bacc
    import concourse.tile as tile
    import concourse.mybir as mybir
    import concourse.bass as bass
    from contextlib import ExitStack

    bf = mybir.dt.bfloat16
    f32 = mybir.dt.float32
    AF = mybir.ActivationFunctionType

    nc = bacc.Bacc("TRN2", target_bir_lowering=False, debug=False)
    # xs: per-macro, 48 rows = 4 groups x 12 (group g lands on partitions
    # 32g..32g+11 so each pair's L1 matmul runs in its own PE row-group and
    # all four execute concurrently via tile_position packing).
    # row layout (group g, d = s*6 + feat): cols 0:256 u0 xt | 256:512 u1 xt
    # | 512:640 u0 wa | 640:768 u1 wa | 768:928 b3both (group 0 row 0 only)
    xw_d = nc.dram_tensor("xw", (teff // 2, 48, XSW), bf, kind="ExternalInput")
    wb_d = nc.dram_tensor("wb", (teff // 2, 128, 472), bf, kind="ExternalInput")
    out_d = nc.dram_tensor("out", (teff // 2, 128, 160), f32,
                           kind="ExternalOutput")

    def bcast_last(ap, n):
        return bass.AP(ap.tensor, ap.offset, list(ap.ap) + [[0, n]])

    with ExitStack() as ctx:
        tc = ctx.enter_context(tile.TileContext(nc))
        cpool = ctx.enter_context(tc.tile_pool(name="const", bufs=1))
        inp = ctx.enter_context(tc.tile_pool(name="inp", bufs=6))
        zpool = ctx.enter_context(tc.tile_pool(name="z", bufs=2))
        spool = ctx.enter_context(tc.tile_pool(name="soft", bufs=2))
        pp1 = ctx.enter_context(tc.tile_pool(name="pp1", bufs=1, space="PSUM"))
        pp2 = ctx.enter_context(tc.tile_pool(name="pp2", bufs=3, space="PSUM"))
        pp3 = ctx.enter_context(tc.tile_pool(name="pp3", bufs=1, space="PSUM"))

        ones_col = cpool.tile([1, 128], bf)
        nc.vector.memset(ones_col[:], 1.0)

        # single PSUM allocations, double-buffered by column halves: L1 psum
        # bank p serves sub-tile u at cols p*512 + u*256 (so L1(u1) never
        # waits on relu1(u0)); p3 alternates 160-col halves per macro.
        p1 = pp1.tile([128, 2048], f32)
        p3t = pp3.tile([128, 320], f32)

        # two supertiles per macro-iteration: DMAs and softmax tail batched
        for m in range(teff // 2):
            xta = inp.tile([128, XSW], bf, tag="xw")
            for g in range(4):
                nc.sync.dma_start(xta[32 * g:32 * g + 12, :],
                                  xw_d.ap()[m, 12 * g:12 * g + 12])
            wb = inp.tile([128, 472], bf, tag="wb")
            nc.sync.dma_start(wb[:], wb_d.ap()[m])

            exps = spool.tile([128, 160], f32, tag="exps")
            p3 = p3t[:, (m % 2) * 160:(m % 2) * 160 + 160]
            # one bias matmul for both sub-tiles: rhs gathers the two b3 rows
            nc.tensor.matmul(p3, ones_col[:], xta[0:1, 768:928],
                             start=True, stop=False)
            z2s = []
            for u in range(2):
                # sub-tile u0 also carries the shared q6 block (s0 = u0's q6,
                # s1 = u1's q6) as its 4th pair; u1 has only 3 pairs.
                npair = 4 if u == 0 else 3
                width = npair * ST
                wbo = u * 300
                # L1: pair p runs in PE row-group p (stationary + moving both
                # live on partitions 32p..32p+11) writing its own PSUM bank,
                # so all npair matmuls execute concurrently.
                uo = u * ST
                for p in range(npair):
                    lhs = xta[32 * p:32 * p + 12,
                              512 + u * 128: 512 + (u + 1) * 128]
                    rhs = xta[32 * p:32 * p + 12, u * ST:(u + 1) * ST]
                    nc.tensor.matmul(
                        p1[:, p * 512 + uo:p * 512 + uo + ST], lhs, rhs,
                        start=True, stop=True, tile_position=(32 * p, 0))
                z1 = zpool.tile([128, width], bf, tag="z1")
                # relu1 in two halves so L2's first chunk starts after only
                # half the DVE latency (subtile deps track the col ranges)
                nhi = npair - 2
                p1v = bass.AP(p1[:].tensor, p1[:].offset + uo,
                              [list(p1[:].ap[0]), [512, 2], [1, ST]])
                z1v = bass.AP(z1[:].tensor, z1[:].offset,
                              [list(z1[:].ap[0]), [ST, 2], [1, ST]])
                nc.vector.tensor_scalar_max(z1v, p1v, 0.0)
                p1h = bass.AP(p1[:].tensor, p1[:].offset + 1024 + uo,
                              [list(p1[:].ap[0]), [512, nhi], [1, ST]])
                z1h = bass.AP(z1[:].tensor, z1[:].offset + 512,
                              [list(z1[:].ap[0]), [ST, nhi], [1, ST]])
                nc.vector.tensor_scalar_max(z1h, p1h, 0.0)

                # L2 in 1-bank psum chunks so relu2/L3 pipeline per chunk
                z2 = zpool.tile([128, width], bf, tag="z2")
                b2ap = wb[:, wbo + 168:wbo + 170].bitcast(f32)
                p2a = pp2.tile([128, 512], f32, tag="p2")
                nc.tensor.matmul(
                    p2a[:], wb[:, wbo:wbo + 128], z1[:, 0:512],
                    start=True, stop=True)
                nc.scalar.activation(z2[:, 0:512], p2a[:],
                                     AF.Relu, bias=b2ap, scale=1.0)
                p2b = pp2.tile([128, 512], f32, tag="p2")
                nc.tensor.matmul(
                    p2b[:, 0:256], wb[:, wbo:wbo + 128], z1[:, 512:768],
                    start=True, stop=True)
                if u == 0:
                    # q6 stationary: dedicated [[W2(j0),0],[0,W2(j1)]] block
                    nc.tensor.matmul(
                        p2b[:, 256:512], wb[:, 172:300],
                        z1[:, 3 * ST:4 * ST],
                        start=True, stop=True)
                    nc.scalar.activation(z2[:, 512:768], p2b[:, 0:256],
                                         AF.Relu, bias=b2ap, scale=1.0)
                    # q6 block needs the mixed bias [b2(j0); b2(j1)]
                    b2mix = wb[:, 170:172].bitcast(f32)
                    nc.scalar.activation(z2[:, 768:1024], p2b[:, 256:512],
                                         AF.Relu, bias=b2mix, scale=1.0)
                else:
                    nc.scalar.activation(z2[:, 512:768], p2b[:, 0:256],
                                         AF.Relu, bias=b2ap, scale=1.0)
                z2s.append(z2)

                # L3 per-(pair, block) products accumulate onto the bias
                for p in range(3):
                    for blk in range(2):
                        nc.tensor.matmul(
                            p3[:, u * 80 + blk * 40 + p * 10:
                               u * 80 + blk * 40 + p * 10 + 10],
                            z2[:, p * ST + blk * 128: p * ST + blk * 128 + 128],
                            wb[:, wbo + 128 + p * 10: wbo + 128 + p * 10 + 10],
                            start=False, stop=False)
            # shared q6 products: one MM per blk, two-range out AP lands
            # s=0 cols in u0's q6 slot and s=1 cols in u1's
            for blk in range(2):
                o3 = p3[:, blk * 40 + 30: blk * 40 + 35]
                out2 = bass.AP(o3.tensor, o3.offset,
                               [list(o3.ap[0]), [80, 2], [1, 5]])
                nc.tensor.matmul(
                    out2,
                    z2s[0][:, 3 * ST + blk * 128: 3 * ST + blk * 128 + 128],
                    wb[:, 128 + 30: 128 + 40],
                    start=False, stop=(blk == 1))
            nc.scalar.activation(exps[:], p3, AF.Exp)

            # softmax tail over both supertiles at once
            den = spool.tile([128, 32], f32, tag="den")
            nc.vector.reduce_sum(
                den[:], exps[:].rearrange("p (g o) -> p g o", o=5),
                axis=mybir.AxisListType.X)
            rden = spool.tile([128, 32], f32, tag="rden")
            nc.vector.reciprocal_approx_fast(rden[:], den[:])
            probs = spool.tile([128, 160], f32, tag="probs")
            nc.gpsimd.tensor_tensor(
                probs[:].rearrange("p (g o) -> p g o", o=5),
                exps[:].rearrange("p (g o) -> p g o", o=5),
                bcast_last(rden[:], 5),
                mybir.AluOpType.mult)

            # last store goes on the SP HWDGE ring: no loads follow it, and
            # it spares the exit drain the slow SWDGE completion wait
            eng = nc.sync if m == teff // 2 - 1 else nc.gpsimd
            eng.dma_start(out_d.ap()[m], probs[:])

    nc.compile()
    return nc


def _get_program(teff=T):
    if teff not in _cache:
        _cache[teff] = _build_program(teff)
    return _cache[teff]


# ----------------------------------------------------------------------------
# host-side data prep
# ----------------------------------------------------------------------------

def _expert_blobs(W1, W1_a, W2, W2_a, V, V_a):
    """Per-expert blobs (uint16 bf16 bits):
    wan [J,48,128]: L1 stationaries in row-group layout (group g = pair g,
        row s*6+d, col s*64+h). Group 3 holds own q6 twice: rows 0:6 in the
        s=0 half (u0 role) and rows 6:12 in the s=1 half (u1/partner role).
    b3e [J,80]: output-layer bias row, col blk*40 + p*10 + s*5 + o.
    wb16 [J,128,WBW]: L2 block + V + biases (unchanged layout).
    q6v [J,64,5]: partner-role V block for the shared q6 pair."""
    W1c = (W1[None] + W1_a).astype(np.float32)    # [J,Q,H,O+1]
    W2c = (W2[None] + W2_a).astype(np.float32)    # [J,H,H+1]
    Vc = (V[None] + V_a).astype(np.float32)       # [J,Q,O,H+1]

    wan = np.zeros((J, 48, 128), np.float32)
    b3 = np.zeros((J, 80), np.float32)
    for q in range(Q):
        p, s = q // 2, q % 2
        # [J, d, h] <- W1c[:, q] is [J, h, d]
        wan[:, p * 12 + s * 6: p * 12 + s * 6 + 6,
            s * 64:(s + 1) * 64] = W1c[:, q].transpose(0, 2, 1)
        for blk in range(2):
            b3[:, blk * 40 + p * 10 + s * 5: blk * 40 + p * 10 + s * 5 + 5] = \
                Vc[:, q, :, 0]
    # own q6 in the partner (s=1) role: group 3 rows 6:12, cols 64:128
    wan[:, 42:48, 64:128] = W1c[:, 6].transpose(0, 2, 1)
    wan16 = wan.astype(_bf16).view(np.uint16)
    b3e = b3.astype(_bf16).view(np.uint16)

    wb = np.zeros((J, 128, 168), np.float32)
    w2w = W2c[:, :, 1:].transpose(0, 2, 1)        # [J, i, h2]
    for s in range(2):
        wb[:, s * 64:(s + 1) * 64, s * 64:(s + 1) * 64] = w2w
    for q in range(Q):
        p, s = q // 2, q % 2
        # [J, h2, o] <- Vc[:, q, :, 1:] is [J, o, h2]
        wb[:, s * 64:(s + 1) * 64, 128 + p * 10 + s * 5: 128 + p * 10 + s * 5 + 5] = \
            Vc[:, q, :, 1:].transpose(0, 2, 1)
    wb16 = np.zeros((J, 128, WBW), np.uint16)
    wb16[:, :, :168] = wb.astype(_bf16).view(np.uint16)
    wb16[:, :, 172:300] = wb16[:, :, 0:128]
    b2 = np.concatenate([W2c[:, :, 0], W2c[:, :, 0]], axis=1)  # [J, 128]
    wb16[:, :, 168:170] = b2.astype(np.float32).view(np.uint16).reshape(J, 128, 2)
    q6v = Vc[:, 6, :, 1:].transpose(0, 2, 1).astype(_bf16).view(np.uint16)  # [J,64,5]
    return wan16, b3e, wb16, q6v


def _plan(judge_ids):
    """Supertile schedule: list of (judge, sample_idx_array), core/slot map."""
    jid = np.asarray(judge_ids).astype(np.int64).ravel()
    assert jid.shape[0] == B
    order = np.argsort(jid, kind="stable")
    counts = np.bincount(jid, minlength=J)
    tiles = []
    pos = 0
    for j in range(J):
        g = order[pos:pos + counts[j]]
        pos += counts[j]
        for s in range(0, len(g), ST):
            tiles.append((j, g[s:s + ST]))
    assert len(tiles) <= N_CORES * T, f"{len(tiles)} supertiles > capacity"
    return tiles


def _prepare_inputs(x, judge_ids, W1, W1_a, W2, W2_a, V, V_a):
    x = np.ascontiguousarray(np.asarray(x, dtype=np.float32))
    wan16, b3e, wb16, q6v = _expert_blobs(*(np.asarray(a, dtype=np.float32)
                                            for a in (W1, W1_a, W2, W2_a, V, V_a)))
    tiles = _plan(judge_ids)
    # compile/run for the actual slot count (rounded to whole macros),
    # not the worst-case T
    teff = -(-len(tiles) // N_CORES)
    teff += teff % 2

    judge_mat = np.zeros((N_CORES, teff), np.int64)      # expert per slot
    xg = np.zeros((N_CORES, teff, ST, Q, O), np.float32)  # gathered x
    for i, (j, g) in enumerate(tiles):
        k, t = i % N_CORES, i // N_CORES
        judge_mat[k, t] = j
        xg[k, t, :len(g)] = x[g]

    # xs stream [48, XSW]: group g rows 12g..12g+11 -> partitions 32g..32g+11
    nm = teff // 2
    xt = np.zeros((N_CORES, nm, 2, 48, ST), np.float32)   # xt per sub-tile u
    for g in range(4):
        xt[:, :, :, g * 12 + 0, :] = 1.0
        xt[:, :, :, g * 12 + 6, :] = 1.0
    xgm = xg.reshape(N_CORES, nm, 2, ST, Q, O)
    for q in range(Q):
        p, s = q // 2, q % 2
        r0 = p * 12 + s * 6
        xt[:, :, :, r0 + 1:r0 + 6, :] = xgm[..., q, :].transpose(0, 1, 2, 4, 3)
    # shared q6 block rides in u0: u1's q6 goes to group 3 rows 7:12 of u0
    xt[:, :, 0, 43:48, :] = xgm[:, :, 1, :, 6, :].transpose(0, 1, 3, 2)
    xt16 = xt.astype(_bf16).view(np.uint16)

    j0 = judge_mat[:, 0::2]   # experts of the two sub-tiles per macro
    j1 = judge_mat[:, 1::2]
    in_maps = []
    for k in range(N_CORES):
        xwm = np.zeros((nm, 48, XSW), np.uint16)
        xwm[:, :, 0:ST] = xt16[k, :, 0]
        xwm[:, :, ST:2 * ST] = xt16[k, :, 1]
        xwm[:, :, 512:640] = wan16[j0[k]]
        # partner's q6 rows into the mixed group-3 stationary (u0 slot)
        xwm[:, 42:48, 576:640] = wan16[j1[k]][:, 42:48, 64:128]
        xwm[:, :, 640:768] = wan16[j1[k]]
        xwm[:, 0, 768:848] = b3e[j0[k]]
        xwm[:, 0, 848:928] = b3e[j1[k]]

        wbs = wb16[judge_mat[k]].copy()
        # partner's q6 V block into the s=1 half of u0's p3 slot
        wbs[0::2, 64:128, 163:168] = q6v[j1[k]]
        # mixed L2 bias [b2(j0); b2(j1)] for the q6 block, f32 bits
        wbs[0::2, 64:128, 170:172] = wbs[1::2, 64:128, 168:170]
        wbs[0::2, 0:64, 170:172] = wbs[0::2, 0:64, 168:170]
        # partner's W2 into the s=1 half of the mixed block
        wbs[0::2, 64:128, 236:300] = wbs[1::2, 64:128, 64:128]
        wbm = np.zeros((nm, 128, 472), np.uint16)
        wbm[:, :, 0:300] = wbs[0::2]
        wbm[:, :, 300:472] = wbs[1::2, :, 0:172]
        in_maps.append({
            "xw": np.ascontiguousarray(xwm).view(_bf16),
            "wb": np.ascontiguousarray(wbm).view(_bf16),
        })
    return in_maps, tiles, teff


def _assemble_output(results, tiles):
    out = np.empty((B, Q, O), np.float32)
    for i, (_, g) in enumerate(tiles):
        k, t = i % N_CORES, i // N_CORES
        blob = results[k]["out"][t // 2][:, (t % 2) * 80:(t % 2) * 80 + 80]
        rows = blob.reshape(128, 2, 40).transpose(1, 0, 2).reshape(ST, 40)
        out[g] = rows[:len(g), :35].reshape(len(g), Q, O)
    return out


# ----------------------------------------------------------------------------
# entry point
# ----------------------------------------------------------------------------

def kernel(x, judge_ids, W1, W1_a, W2, W2_a, V, V_a):
    from concourse import bass_utils
    in_maps, tiles, teff = _prepare_inputs(x, judge_ids, W1, W1_a, W2, W2_a, V, V_a)
    nc = _get_program(teff)
    res = bass_utils.run_bass_kernel_spmd(
        nc, in_maps, core_ids=list(range(N_CORES)), trace=False)
    return _assemble_output(res.results, tiles)


# expose for test harness reuse
def run_with_results(x, judge_ids, W1, W1_a, W2, W2_a, V, V_a, trace=False,
                     **kwargs):
    from concourse import bass_utils
    in_maps, tiles, teff = _prepare_inputs(x, judge_ids, W1, W1_a, W2, W2_a, V, V_a)
    nc = _get_program(teff)
    res = bass_utils.run_bass_kernel_spmd(
        nc, in_maps, core_ids=list(range(N_CORES)), trace=trace, **kwargs)
    return _assemble_output(res.results, tiles), res



# revision 23
# speedup vs baseline: 1.0008x; 1.0008x over previous
"""Trainium2 Bass kernel for nn_CalibrationNetwork (MoE-routed 3-layer MLP + softmax).

Strategy
--------
Host side (numpy): sort samples by judge id, pad each judge group to a
multiple of 256 samples ("supertile"), distribute supertiles round-robin
over 8 cores (20 slots/core covers the worst case sum_j ceil(c_j/256) <= 160).
All judge-specific tables are combined with the shared ones on the host
(W1+W1_a[j] etc.), so the device never gathers. Questions are processed as
block-diagonal pairs (two 64-wide blocks fill the 128 partitions/PE columns);
the odd 7th question (q6) of the two supertiles in a macro-iteration shares
one block (s=0 half = first tile's q6, s=1 half = second tile's q6, with the
two tiles' experts mixed in the block-diagonal weights).

Per macro (two supertiles) the host emits two HBM streams:
  - xs [48, 928] bf16: 4 groups of 12 rows; group g lands on SBUF partitions
    32g..32g+11 so pair g's L1 matmul runs in PE row-group g and all four
    execute concurrently (tile_position packing). Row (g, s*6+d): cols 0:256
    u0 x^T (d=0 ones), 256:512 u1 x^T, 512:640 u0 L1 stationary (col s*64+h),
    640:768 u1 stationary, 768:928 both tiles' b3 rows (group-0 row 0).
  - wb [128, 472] bf16: per sub-tile 300 cols: L2 block-diag stationary
    [[W2,0],[0,W2]], V moving operands, f32-bit bias cols, q6-mixed L2 block.

Device (per macro; all engines pipelined by Tile):
  bias: 1 matmul ones_col^T @ b3rows -> psum3 [128, 160]        (K=1, N=160)
  L1: 7 matmuls  psum1[(s,h1), b] += wa_p^T @ xt_p  (K=12, N=256, row-packed:
      pair p -> PSUM bank p cols u*256, so the 4 pairs run concurrently and
      sub-tiles u0/u1 double-buffer by column halves within one allocation)
  relu1 (DVE, 2 ops): z1 = max(psum1, 0) -> bf16
  L2: 1-bank psum chunks [128, 512]: N=512 + N=256(+q6 N=256), relu2 (ACT)
      per chunk so L3 starts after only part of the ACT latency
  L3: 16 matmuls psum3[b, cols] += z2_slice^T @ vb_p            (K=128, N=5..10)
  softmax: exp (ACT), grouped reduce_sum + reciprocal (DVE), multiply (GpSimd)
  p3 psum alternates 160-col halves of one bank per macro; the [128, 160]
  f32 store goes out as one 128-descriptor DMA on the GpSimd SWDGE queue.

The program is compiled per call for the actual supertile count
(rounded to whole macros) rather than the worst-case T=20; with the
reference inputs that is 18 slots/core (9 macros).
Measured on 8 axon NeuronCores: ~56-60 us HW exec (device clock state
varies ~15% run to run), absmax err ~1.1e-3.
"""

import numpy as np
import ml_dtypes

B, J, Q, O = 32768, 32, 7, 5
H = 64            # H1 == H2
ST = 256          # samples per supertile
T = 20            # supertiles per core (worst case 32768/256 + 32 = 160 = 8*20)
N_CORES = 8
QP = 4            # question pairs (Q=7 padded to 8)
XSW = 928           # xs row width: 2*256 xt | 2*128 wa | 160 b3 (row 0)
WBW = 300           # per-tile wb width (172 + mixed-W2 block 128)

_bf16 = ml_dtypes.bfloat16
_cache = {}


# ----------------------------------------------------------------------------
# device program
# ----------------------------------------------------------------------------

def _build_program(teff):
    import concourse.bacc as bacc
    import concourse.tile as tile
    import concourse.mybir as mybir
    import concourse.bass as bass
    from contextlib import ExitStack

    bf = mybir.dt.bfloat16
    f32 = mybir.dt.float32
    AF = mybir.ActivationFunctionType

    nc = bacc.Bacc("TRN2", target_bir_lowering=False, debug=False)
    # xs: per-macro, 48 rows = 4 groups x 12 (group g lands on partitions
    # 32g..32g+11 so each pair's L1 matmul runs in its own PE row-group and
    # all four execute concurrently via tile_position packing).
    # row layout (group g, d = s*6 + feat): cols 0:256 u0 xt | 256:512 u1 xt
    # | 512:640 u0 wa | 640:768 u1 wa | 768:928 b3both (group 0 row 0 only)
    xw_d = nc.dram_tensor("xw", (teff // 2, 48, XSW), bf, kind="ExternalInput")
    wb_d = nc.dram_tensor("wb", (teff // 2, 128, 472), bf, kind="ExternalInput")
    out_d = nc.dram_tensor("out", (teff // 2, 128, 160), f32,
                           kind="ExternalOutput")

    def bcast_last(ap, n):
        return bass.AP(ap.tensor, ap.offset, list(ap.ap) + [[0, n]])

    with ExitStack() as ctx:
        tc = ctx.enter_context(tile.TileContext(nc))
        cpool = ctx.enter_context(tc.tile_pool(name="const", bufs=1))
        inp = ctx.enter_context(tc.tile_pool(name="inp", bufs=6))
        zpool = ctx.enter_context(tc.tile_pool(name="z", bufs=3))
        spool = ctx.enter_context(tc.tile_pool(name="soft", bufs=3))
        pp1 = ctx.enter_context(tc.tile_pool(name="pp1", bufs=1, space="PSUM"))
        pp2 = ctx.enter_context(tc.tile_pool(name="pp2", bufs=3, space="PSUM"))
        pp3 = ctx.enter_context(tc.tile_pool(name="pp3", bufs=1, space="PSUM"))

        ones_col = cpool.tile([1, 128], bf)
        nc.vector.memset(ones_col[:], 1.0)

        # single PSUM allocations, double-buffered by column halves: L1 psum
        # bank p serves sub-tile u at cols p*512 + u*256 (so L1(u1) never
        # waits on relu1(u0)); p3 alternates 160-col halves per macro.
        p1 = pp1.tile([128, 2048], f32)
        p3t = pp3.tile([128, 320], f32)

        # two supertiles per macro-iteration: DMAs and softmax tail batched
        for m in range(teff // 2):
            xta = inp.tile([128, XSW], bf, tag="xw")
            for g in range(4):
                nc.sync.dma_start(xta[32 * g:32 * g + 12, :],
                                  xw_d.ap()[m, 12 * g:12 * g + 12])
            wb = inp.tile([128, 472], bf, tag="wb")
            nc.sync.dma_start(wb[:], wb_d.ap()[m])

            exps = spool.tile([128, 160], f32, tag="exps")
            p3 = p3t[:, (m % 2) * 160:(m % 2) * 160 + 160]
            # one bias matmul for both sub-tiles: rhs gathers the two b3 rows
            nc.tensor.matmul(p3, ones_col[:], xta[0:1, 768:928],
                             start=True, stop=False)
            z2s = []
            for u in range(2):
                # sub-tile u0 also carries the shared q6 block (s0 = u0's q6,
                # s1 = u1's q6) as its 4th pair; u1 has only 3 pairs.
                npair = 4 if u == 0 else 3
                width = npair * ST
                wbo = u * 300
                # L1: pair p runs in PE row-group p (stationary + moving both
                # live on partitions 32p..32p+11) writing its own PSUM bank,
                # so all npair matmuls execute concurrently.
                uo = u * ST
                for p in range(npair):
                    lhs = xta[32 * p:32 * p + 12,
                              512 + u * 128: 512 + (u + 1) * 128]
                    rhs = xta[32 * p:32 * p + 12, u * ST:(u + 1) * ST]
                    nc.tensor.matmul(
                        p1[:, p * 512 + uo:p * 512 + uo + ST], lhs, rhs,
                        start=True, stop=True, tile_position=(32 * p, 0))
                z1 = zpool.tile([128, width], bf, tag="z1")
                # relu1 in two halves so L2's first chunk starts after only
                # half the DVE latency (subtile deps track the col ranges)
                nhi = npair - 2
                p1v = bass.AP(p1[:].tensor, p1[:].offset + uo,
                              [list(p1[:].ap[0]), [512, 2], [1, ST]])
                z1v = bass.AP(z1[:].tensor, z1[:].offset,
                              [list(z1[:].ap[0]), [ST, 2], [1, ST]])
                nc.vector.tensor_scalar_max(z1v, p1v, 0.0)
                p1h = bass.AP(p1[:].tensor, p1[:].offset + 1024 + uo,
                              [list(p1[:].ap[0]), [512, nhi], [1, ST]])
                z1h = bass.AP(z1[:].tensor, z1[:].offset + 512,
                              [list(z1[:].ap[0]), [ST, nhi], [1, ST]])
                nc.vector.tensor_scalar_max(z1h, p1h, 0.0)

                # L2 in 1-bank psum chunks so relu2/L3 pipeline per chunk
                z2 = zpool.tile([128, width], bf, tag="z2")
                b2ap = wb[:, wbo + 168:wbo + 170].bitcast(f32)
                p2a = pp2.tile([128, 512], f32, tag="p2")
                nc.tensor.matmul(
                    p2a[:], wb[:, wbo:wbo + 128], z1[:, 0:512],
                    start=True, stop=True)
                nc.scalar.activation(z2[:, 0:512], p2a[:],
                                     AF.Relu, bias=b2ap, scale=1.0)
                p2b = pp2.tile([128, 512], f32, tag="p2")
                nc.tensor.matmul(
                    p2b[:, 0:256], wb[:, wbo:wbo + 128], z1[:, 512:768],
                    start=True, stop=True)
                if u == 0:
                    # q6 stationary: dedicated [[W2(j0),0],[0,W2(j1)]] block
                    nc.tensor.matmul(
                        p2b[:, 256:512], wb[:, 172:300],
                        z1[:, 3 * ST:4 * ST],
                        start=True, stop=True)
                    nc.scalar.activation(z2[:, 512:768], p2b[:, 0:256],
                                         AF.Relu, bias=b2ap, scale=1.0)
                    # q6 block needs the mixed bias [b2(j0); b2(j1)]
                    b2mix = wb[:, 170:172].bitcast(f32)
                    nc.scalar.activation(z2[:, 768:1024], p2b[:, 256:512],
                                         AF.Relu, bias=b2mix, scale=1.0)
                else:
                    nc.scalar.activation(z2[:, 512:768], p2b[:, 0:256],
                                         AF.Relu, bias=b2ap, scale=1.0)
                z2s.append(z2)

                # L3 per-(pair, block) products accumulate onto the bias
                for p in range(3):
                    for blk in range(2):
                        nc.tensor.matmul(
                            p3[:, u * 80 + blk * 40 + p * 10:
                               u * 80 + blk * 40 + p * 10 + 10],
                            z2[:, p * ST + blk * 128: p * ST + blk * 128 + 128],
                            wb[:, wbo + 128 + p * 10: wbo + 128 + p * 10 + 10],
                            start=False, stop=False)
            # shared q6 products: one MM per blk, two-range out AP lands
            # s=0 cols in u0's q6 slot and s=1 cols in u1's
            for blk in range(2):
                o3 = p3[:, blk * 40 + 30: blk * 40 + 35]
                out2 = bass.AP(o3.tensor, o3.offset,
                               [list(o3.ap[0]), [80, 2], [1, 5]])
                nc.tensor.matmul(
                    out2,
                    z2s[0][:, 3 * ST + blk * 128: 3 * ST + blk * 128 + 128],
                    wb[:, 128 + 30: 128 + 40],
                    start=False, stop=(blk == 1))
            nc.scalar.activation(exps[:], p3, AF.Exp)

            # softmax tail over both supertiles at once
            den = spool.tile([128, 32], f32, tag="den")
            nc.vector.reduce_sum(
                den[:], exps[:].rearrange("p (g o) -> p g o", o=5),
                axis=mybir.AxisListType.X)
            rden = spool.tile([128, 32], f32, tag="rden")
            nc.vector.reciprocal_approx_fast(rden[:], den[:])
            probs = spool.tile([128, 160], f32, tag="probs")
            nc.gpsimd.tensor_tensor(
                probs[:].rearrange("p (g o) -> p g o", o=5),
                exps[:].rearrange("p (g o) -> p g o", o=5),
                bcast_last(rden[:], 5),
                mybir.AluOpType.mult)

            # last store goes on the SP HWDGE ring: no loads follow it, and
            # it spares the exit drain the slow SWDGE completion wait
            eng = nc.sync if m == teff // 2 - 1 else nc.gpsimd
            eng.dma_start(out_d.ap()[m], probs[:])

    nc.compile()
    return nc


def _get_program(teff=T):
    if teff not in _cache:
        _cache[teff] = _build_program(teff)
    return _cache[teff]


# ----------------------------------------------------------------------------
# host-side data prep
# ----------------------------------------------------------------------------

def _expert_blobs(W1, W1_a, W2, W2_a, V, V_a):
    """Per-expert blobs (uint16 bf16 bits):
    wan [J,48,128]: L1 stationaries in row-group layout (group g = pair g,
        row s*6+d, col s*64+h). Group 3 holds own q6 twice: rows 0:6 in the
        s=0 half (u0 role) and rows 6:12 in the s=1 half (u1/partner role).
    b3e [J,80]: output-layer bias row, col blk*40 + p*10 + s*5 + o.
    wb16 [J,128,WBW]: L2 block + V + biases (unchanged layout).
    q6v [J,64,5]: partner-role V block for the shared q6 pair."""
    W1c = (W1[None] + W1_a).astype(np.float32)    # [J,Q,H,O+1]
    W2c = (W2[None] + W2_a).astype(np.float32)    # [J,H,H+1]
    Vc = (V[None] + V_a).astype(np.float32)       # [J,Q,O,H+1]

    wan = np.zeros((J, 48, 128), np.float32)
    b3 = np.zeros((J, 80), np.float32)
    for q in range(Q):
        p, s = q // 2, q % 2
        # [J, d, h] <- W1c[:, q] is [J, h, d]
        wan[:, p * 12 + s * 6: p * 12 + s * 6 + 6,
            s * 64:(s + 1) * 64] = W1c[:, q].transpose(0, 2, 1)
        for blk in range(2):
            b3[:, blk * 40 + p * 10 + s * 5: blk * 40 + p * 10 + s * 5 + 5] = \
                Vc[:, q, :, 0]
    # own q6 in the partner (s=1) role: group 3 rows 6:12, cols 64:128
    wan[:, 42:48, 64:128] = W1c[:, 6].transpose(0, 2, 1)
    wan16 = wan.astype(_bf16).view(np.uint16)
    b3e = b3.astype(_bf16).view(np.uint16)

    wb = np.zeros((J, 128, 168), np.float32)
    w2w = W2c[:, :, 1:].transpose(0, 2, 1)        # [J, i, h2]
    for s in range(2):
        wb[:, s * 64:(s + 1) * 64, s * 64:(s + 1) * 64] = w2w
    for q in range(Q):
        p, s = q // 2, q % 2
        # [J, h2, o] <- Vc[:, q, :, 1:] is [J, o, h2]
        wb[:, s * 64:(s + 1) * 64, 128 + p * 10 + s * 5: 128 + p * 10 + s * 5 + 5] = \
            Vc[:, q, :, 1:].transpose(0, 2, 1)
    wb16 = np.zeros((J, 128, WBW), np.uint16)
    wb16[:, :, :168] = wb.astype(_bf16).view(np.uint16)
    wb16[:, :, 172:300] = wb16[:, :, 0:128]
    b2 = np.concatenate([W2c[:, :, 0], W2c[:, :, 0]], axis=1)  # [J, 128]
    wb16[:, :, 168:170] = b2.astype(np.float32).view(np.uint16).reshape(J, 128, 2)
    q6v = Vc[:, 6, :, 1:].transpose(0, 2, 1).astype(_bf16).view(np.uint16)  # [J,64,5]
    return wan16, b3e, wb16, q6v


def _plan(judge_ids):
    """Supertile schedule: list of (judge, sample_idx_array), core/slot map."""
    jid = np.asarray(judge_ids).astype(np.int64).ravel()
    assert jid.shape[0] == B
    order = np.argsort(jid, kind="stable")
    counts = np.bincount(jid, minlength=J)
    tiles = []
    pos = 0
    for j in range(J):
        g = order[pos:pos + counts[j]]
        pos += counts[j]
        for s in range(0, len(g), ST):
            tiles.append((j, g[s:s + ST]))
    assert len(tiles) <= N_CORES * T, f"{len(tiles)} supertiles > capacity"
    return tiles


def _prepare_inputs(x, judge_ids, W1, W1_a, W2, W2_a, V, V_a):
    x = np.ascontiguousarray(np.asarray(x, dtype=np.float32))
    wan16, b3e, wb16, q6v = _expert_blobs(*(np.asarray(a, dtype=np.float32)
                                            for a in (W1, W1_a, W2, W2_a, V, V_a)))
    tiles = _plan(judge_ids)
    # compile/run for the actual slot count (rounded to whole macros),
    # not the worst-case T
    teff = -(-len(tiles) // N_CORES)
    teff += teff % 2

    judge_mat = np.zeros((N_CORES, teff), np.int64)      # expert per slot
    xg = np.zeros((N_CORES, teff, ST, Q, O), np.float32)  # gathered x
    for i, (j, g) in enumerate(tiles):
        k, t = i % N_CORES, i // N_CORES
        judge_mat[k, t] = j
        xg[k, t, :len(g)] = x[g]

    # xs stream [48, XSW]: group g rows 12g..12g+11 -> partitions 32g..32g+11
    nm = teff // 2
    xt = np.zeros((N_CORES, nm, 2, 48, ST), np.float32)   # xt per sub-tile u
    for g in range(4):
        xt[:, :, :, g * 12 + 0, :] = 1.0
        xt[:, :, :, g * 12 + 6, :] = 1.0
    xgm = xg.reshape(N_CORES, nm, 2, ST, Q, O)
    for q in range(Q):
        p, s = q // 2, q % 2
        r0 = p * 12 + s * 6
        xt[:, :, :, r0 + 1:r0 + 6, :] = xgm[..., q, :].transpose(0, 1, 2, 4, 3)
    # shared q6 block rides in u0: u1's q6 goes to group 3 rows 7:12 of u0
    xt[:, :, 0, 43:48, :] = xgm[:, :, 1, :, 6, :].transpose(0, 1, 3, 2)
    xt16 = xt.astype(_bf16).view(np.uint16)

    j0 = judge_mat[:, 0::2]   # experts of the two sub-tiles per macro
    j1 = judge_mat[:, 1::2]
    in_maps = []
    for k in range(N_CORES):
        xwm = np.zeros((nm, 48, XSW), np.uint16)
        xwm[:, :, 0:ST] = xt16[k, :, 0]
        xwm[:, :, ST:2 * ST] = xt16[k, :, 1]
        xwm[:, :, 512:640] = wan16[j0[k]]
        # partner's q6 rows into the mixed group-3 stationary (u0 slot)
        xwm[:, 42:48, 576:640] = wan16[j1[k]][:, 42:48, 64:128]
        xwm[:, :, 640:768] = wan16[j1[k]]
        xwm[:, 0, 768:848] = b3e[j0[k]]
        xwm[:, 0, 848:928] = b3e[j1[k]]

        wbs = wb16[judge_mat[k]].copy()
        # partner's q6 V block into the s=1 half of u0's p3 slot
        wbs[0::2, 64:128, 163:168] = q6v[j1[k]]
        # mixed L2 bias [b2(j0); b2(j1)] for the q6 block, f32 bits
        wbs[0::2, 64:128, 170:172] = wbs[1::2, 64:128, 168:170]
        wbs[0::2, 0:64, 170:172] = wbs[0::2, 0:64, 168:170]
        # partner's W2 into the s=1 half of the mixed block
        wbs[0::2, 64:128, 236:300] = wbs[1::2, 64:128, 64:128]
        wbm = np.zeros((nm, 128, 472), np.uint16)
        wbm[:, :, 0:300] = wbs[0::2]
        wbm[:, :, 300:472] = wbs[1::2, :, 0:172]
        in_maps.append({
            "xw": np.ascontiguousarray(xwm).view(_bf16),
            "wb": np.ascontiguousarray(wbm).view(_bf16),
        })
    return in_maps, tiles, teff


def _assemble_output(results, tiles):
    out = np.empty((B, Q, O), np.float32)
    for i, (_, g) in enumerate(tiles):
        k, t = i % N_CORES, i // N_CORES
        blob = results[k]["out"][t // 2][:, (t % 2) * 80:(t % 2) * 80 + 80]
        rows = blob.reshape(128, 2, 40).transpose(1, 0, 2).reshape(ST, 40)
        out[g] = rows[:len(g), :35].reshape(len(g), Q, O)
    return out


# ----------------------------------------------------------------------------
# entry point
# ----------------------------------------------------------------------------

def kernel(x, judge_ids, W1, W1_a, W2, W2_a, V, V_a):
    from concourse import bass_utils
    in_maps, tiles, teff = _prepare_inputs(x, judge_ids, W1, W1_a, W2, W2_a, V, V_a)
    nc = _get_program(teff)
    res = bass_utils.run_bass_kernel_spmd(
        nc, in_maps, core_ids=list(range(N_CORES)), trace=False)
    return _assemble_output(res.results, tiles)


# expose for test harness reuse
def run_with_results(x, judge_ids, W1, W1_a, W2, W2_a, V, V_a, trace=False,
                     **kwargs):
    from concourse import bass_utils
    in_maps, tiles, teff = _prepare_inputs(x, judge_ids, W1, W1_a, W2, W2_a, V, V_a)
    nc = _get_program(teff)
    res = bass_utils.run_bass_kernel_spmd(
        nc, in_maps, core_ids=list(range(N_CORES)), trace=trace, **kwargs)
    return _assemble_output(res.results, tiles), res

